# revision 1
# baseline (speedup 1.0000x reference)
"""V3: v2 + DMA batching (HWDGE issue overhead was the phase-1 bottleneck).

- x block load: one DMA via (kc p) t -> p kc t rearrange (was 16).
- weight load: one DMA (was 16).
- rope rotate-half swap batched across the 4 q/k dim-tiles (2 DMAs/block).
- q spill + v spill: one multi-dim DMA per block each.
- y written in (128, 2048) row blocks (one DMA per token tile), with the
  PSUM->SBUF copies on the scalar engine (DVE was saturating).
"""

import math
from contextlib import ExitStack

import numpy as np

import concourse.bass as bass
import concourse.tile as tile
from concourse import bacc, mybir
from concourse.bass_utils import run_bass_kernel_spmd

B, L, H, NH, HD = 2, 2048, 2048, 16, 128
ROPE_THETA = 10000.0
N_CORES = 8
NH_LOC = NH // N_CORES          # 2
QKV_LOC = 3 * NH_LOC * HD       # 768
D_LOC = NH_LOC * HD             # 256
BL = B * L
P = 128
KC = H // P                     # 16
BLK = 256
NBLK = BL // BLK                # 16
BLK_PER_B = NBLK // B           # 8
QS = 512
NQS = L // QS
KT = L // P
NBH = B * NH_LOC                # 4

F32 = mybir.dt.float32
F32R = mybir.dt.float32r
EXP = mybir.ActivationFunctionType.Exp
NEG = -30000.0


def _build():
    nc = bacc.Bacc("TRN2", target_bir_lowering=False, debug=False,
                   num_devices=N_CORES)

    xT = nc.dram_tensor("xT", [H, BL], F32R, kind="ExternalInput").ap()
    wT = nc.dram_tensor("wT", [H, QKV_LOC], F32R, kind="ExternalInput").ap()
    woT = nc.dram_tensor("woT", [D_LOC, H], F32R, kind="ExternalInput").ap()
    cosT = nc.dram_tensor("cosT", [HD, L], F32, kind="ExternalInput").ap()
    sinTs = nc.dram_tensor("sinTs", [HD, L], F32, kind="ExternalInput").ap()
    tri = nc.dram_tensor("tri", [P, P], F32, kind="ExternalInput").ap()
    ones_in = nc.dram_tensor("ones", [P, P], F32R, kind="ExternalInput").ap()
    y = nc.dram_tensor("y", [BL, H], F32, kind="ExternalOutput").ap()

    qT_d = nc.dram_tensor("qT_d", [NBH, HD, L], F32R).ap()
    vN_d = nc.dram_tensor("vN_d", [NBH, L, HD], F32R).ap()
    oT_d = nc.dram_tensor("oT_d", [NBH, HD, L], F32R).ap()

    with tile.TileContext(nc) as tc, ExitStack() as ctx:
        g = ctx.enter_context(tc.tile_pool(name="g", bufs=1))
        kt_all = g.tile([P, NBH, L], F32R)      # resident rope'd k-cache

        p2c = ctx.enter_context(tc.tile_pool(name="p2c", bufs=1))
        p2q = ctx.enter_context(tc.tile_pool(name="p2q", bufs=2))
        p2v = ctx.enter_context(tc.tile_pool(name="p2v", bufs=1))
        p2e = ctx.enter_context(tc.tile_pool(name="p2e", bufs=6))
        p2t = ctx.enter_context(tc.tile_pool(name="p2t", bufs=2))
        ps2s = ctx.enter_context(tc.tile_pool(name="ps2s", bufs=2, space="PSUM"))
        ps2o = ctx.enter_context(tc.tile_pool(name="ps2o", bufs=1, space="PSUM"))
        ps2d = ctx.enter_context(tc.tile_pool(name="ps2d", bufs=1, space="PSUM"))

        # ---------------- phase 1: QKV projection + RoPE ----------------
        with tc.tile_pool(name="p1w", bufs=1) as p1w, \
             tc.tile_pool(name="p1x", bufs=2) as p1x, \
             tc.tile_pool(name="p1t", bufs=2) as p1t, \
             tc.tile_pool(name="ps1", bufs=2, space="PSUM") as ps1, \
             tc.tile_pool(name="ps1v", bufs=2, space="PSUM") as ps1v:
            wt = p1w.tile([P, KC, QKV_LOC], F32R)
            nc.sync.dma_start(wt[:, 0:2, :],
                              wT[0:2 * P, :].rearrange("(n p) d -> p n d", p=P))
            xb0 = p1x.tile([P, KC, BLK], F32R, name="xb")
            nc.sync.dma_start(xb0[:, 0:4, :],
                              xT[0:4 * P, 0:BLK]
                              .rearrange("(n p) t -> p n t", p=P))
            nc.sync.dma_start(xb0[:, 4:KC, :],
                              xT[4 * P:KC * P, 0:BLK]
                              .rearrange("(n p) t -> p n t", p=P))
            for kq in range(2, KC, 7):
                hi = min(kq + 7, KC)
                nc.sync.dma_start(
                    wt[:, kq:hi, :],
                    wT[kq * P:hi * P, :].rearrange("(n p) d -> p n d", p=P))
            cost = p1w.tile([P, L], F32)
            sints = p1w.tile([P, L], F32)
            for ch in range(4):
                sl = slice(ch * 512, (ch + 1) * 512)
                nc.sync.dma_start(cost[:, sl], cosT[:, sl])
                nc.sync.dma_start(sints[:, sl], sinTs[:, sl])

            for blk in range(NBLK):
                b, lo = divmod(blk, BLK_PER_B)
                lo *= BLK
                col = blk * BLK
                if blk == 0:
                    xb = xb0
                else:
                    xb = p1x.tile([P, KC, BLK], F32R, name="xb")
                    nc.sync.dma_start(
                        xb[:], xT[:, col:col + BLK]
                        .rearrange("(n p) t -> p n t", p=P))

                qc = p1t.tile([P, 4, BLK], F32, name="qc")
                for dt_i in range(4):     # 0,1 = q heads; 2,3 = k heads
                    psum = ps1.tile([P, BLK], F32, name="qkps")
                    for kc in range(KC):
                        nc.tensor.matmul(
                            psum[:], lhsT=wt[:, kc, dt_i * P:dt_i * P + P],
                            rhs=xb[:, kc, :],
                            start=(kc == 0), stop=(kc == KC - 1))
                    nc.vector.tensor_copy(qc[:, dt_i, :], psum[:])
                # batched rotate-half swap for all 4 dim-tiles
                qsw = p1t.tile([P, 4, BLK], F32, name="qsw")
                nc.sync.dma_start(qsw[0:64, :, :], qc[64:128, :, :])
                nc.sync.dma_start(qsw[64:128, :, :], qc[0:64, :, :])

                qr = p1t.tile([P, NH_LOC, BLK], F32R, name="qr")
                for dt_i in range(4):
                    qk, hh = divmod(dt_i, 2)
                    bh = b * NH_LOC + hh
                    t1 = p1t.tile([P, BLK], F32, name="t1")
                    nc.vector.tensor_mul(t1[:], qc[:, dt_i, :],
                                         cost[:, lo:lo + BLK])
                    t2 = p1t.tile([P, BLK], F32, name="t2")
                    nc.vector.tensor_mul(t2[:], qsw[:, dt_i, :],
                                         sints[:, lo:lo + BLK])
                    if qk == 0:
                        nc.vector.tensor_add(qr[:, hh, :], t1[:], t2[:])
                    else:
                        nc.vector.tensor_add(kt_all[:, bh, lo:lo + BLK],
                                             t1[:], t2[:])
                nc.scalar.dma_start(
                    qT_d[b * NH_LOC:(b + 1) * NH_LOC, :, lo:lo + BLK]
                    .rearrange("h p t -> p h t"), qr[:])

                vsb = p1t.tile([P, BLK // P, D_LOC], F32R, name="vsb")
                for tt in range(BLK // P):
                    psv = ps1v.tile([P, D_LOC], F32, name="vps")
                    for kc in range(KC):
                        nc.tensor.matmul(
                            psv[:], lhsT=xb[:, kc, tt * P:(tt + 1) * P],
                            rhs=wt[:, kc, 2 * D_LOC:3 * D_LOC],
                            start=(kc == 0), stop=(kc == KC - 1))
                    nc.vector.tensor_copy(vsb[:, tt, :], psv[:])
                # one DMA per head covering both token tiles of this block
                for hh in range(NH_LOC):
                    nc.scalar.dma_start(
                        vN_d[b * NH_LOC + hh, lo:lo + BLK, :]
                        .rearrange("(n p) d -> p n d", p=P),
                        vsb[:, :, hh * HD:(hh + 1) * HD])

        # ---------------- phase 2: causal attention ----------------
        trimask = p2c.tile([P, P], F32)
        nc.sync.dma_start(trimask[:], tri[:])
        ones = p2c.tile([P, P], F32R)
        nc.sync.dma_start(ones[:], ones_in[:])

        otr_ctx = ExitStack()
        for bh in range(NBH):
            if bh == 2:
                otr_pool = otr_ctx.enter_context(
                    tc.tile_pool(name="otr", bufs=1))
                ot_res = otr_pool.tile([P, NH_LOC, L], F32R)
            qt = p2q.tile([P, L], F32R, name="qt")
            vn = p2v.tile([P, KT, HD], F32R, name="vn")
            for qs_i in range(NQS):
                qs = qs_i * QS
                nkt = (qs + QS) // P
                # chunked loads: this q-slice only needs phase-1 output up
                # to token qs+QS, so attention starts while phase 1 runs
                nc.sync.dma_start(qt[:, qs:qs + QS], qT_d[bh][:, qs:qs + QS])
                nc.sync.dma_start(
                    vn[:, nkt - QS // P:nkt, :],
                    vN_d[bh][qs:qs + QS, :].rearrange("(n p) d -> p n d", p=P))
                po = ps2o.tile([P, QS], F32, name="po")
                pd = ps2d.tile([P, QS], F32, name="pd")
                for k_i in range(nkt):
                    d = k_i * P - qs
                    c0 = max(d, 0)
                    psc = ps2s.tile([P, QS], F32, name="psc")
                    nc.tensor.matmul(
                        psc[:, c0:QS],
                        lhsT=kt_all[:, bh, k_i * P:(k_i + 1) * P],
                        rhs=qt[:, qs + c0:qs + QS],
                        start=True, stop=True)
                    et = p2e.tile([P, QS], F32R, name="et")
                    if d >= 0:
                        smsm = p2t.tile([P, P], F32, name="smsm")
                        nc.vector.tensor_add(smsm[:], psc[:, d:d + P],
                                             trimask[:])
                        nc.scalar.activation(et[:, d:d + P], smsm[:], EXP)
                        if d + P < QS:
                            nc.scalar.activation(et[:, d + P:QS],
                                                 psc[:, d + P:QS], EXP)
                    else:
                        nc.scalar.activation(et[:, 0:QS], psc[:, 0:QS], EXP)
                    nc.tensor.matmul(po[:, c0:QS], lhsT=vn[:, k_i, :],
                                     rhs=et[:, c0:QS], start=(k_i == 0),
                                     stop=(k_i == nkt - 1))
                    nc.tensor.matmul(pd[:, c0:QS], lhsT=ones[:],
                                     rhs=et[:, c0:QS], start=(k_i == 0),
                                     stop=(k_i == nkt - 1))
                rec = p2t.tile([P, QS], F32, name="rec")
                nc.vector.reciprocal(rec[:], pd[:])
                if bh < 2:
                    ot = p2t.tile([P, QS], F32R, name="ot")
                    nc.vector.tensor_mul(ot[:], po[:], rec[:])
                    nc.scalar.dma_start(oT_d[bh, :, qs:qs + QS], ot[:])
                else:
                    nc.vector.tensor_mul(ot_res[:, bh - 2, qs:qs + QS],
                                         po[:], rec[:])

        # ---------------- phase 3: output projection (partial) ----------------
        with tc.tile_pool(name="p3w", bufs=1) as p3w, \
             tc.tile_pool(name="p3b", bufs=2) as p3b, \
             tc.tile_pool(name="p3y", bufs=3) as p3y, \
             tc.tile_pool(name="ps3", bufs=2, space="PSUM") as ps3:
            wo = p3w.tile([P, NH_LOC, H], F32R)
            for hh in range(NH_LOC):
                nc.sync.dma_start(wo[:, hh, :], woT[hh * P:(hh + 1) * P, :])
            for b in range(B):
                if b == 0:
                    otb = p3b.tile([P, NH_LOC, L], F32R, name="otb")
                    for hh in range(NH_LOC):
                        for qi in range(NQS):
                            sl = slice(qi * QS, (qi + 1) * QS)
                            nc.sync.dma_start(otb[:, hh, sl],
                                              oT_d[hh][:, sl])
                else:
                    otb = ot_res
                for tt in range(L // P):
                    ybig = p3y.tile([P, H], F32, name="ybig")
                    for oc in range(H // 512):
                        py_ = ps3.tile([P, 512], F32, name="py")
                        for hh in range(NH_LOC):
                            nc.tensor.matmul(
                                py_[:],
                                lhsT=otb[:, hh, tt * P:(tt + 1) * P],
                                rhs=wo[:, hh, oc * 512:(oc + 1) * 512],
                                start=(hh == 0), stop=(hh == NH_LOC - 1))
                        nc.vector.tensor_copy(
                            ybig[:, oc * 512:(oc + 1) * 512], py_[:])
                    for half in range(2):
                        hs = slice(half * (H // 2), (half + 1) * (H // 2))
                        nc.sync.dma_start(
                            y[b * L + tt * P: b * L + (tt + 1) * P, hs],
                            ybig[:, hs])

        otr_ctx.close()

    nc.compile()
    return nc


_NC = None


def _get_nc():
    global _NC
    if _NC is None:
        _NC = _build()
    return _NC


def _host_inputs(x, Wqkv, Wo):
    x = np.asarray(x, dtype=np.float32)
    Wqkv = np.asarray(Wqkv, dtype=np.float32)
    Wo = np.asarray(Wo, dtype=np.float32)

    xT = np.ascontiguousarray(x.reshape(BL, H).T)

    inv_freq = 1.0 / (ROPE_THETA ** (np.arange(0, HD, 2, dtype=np.float32)
                                     / HD))
    t = np.arange(L, dtype=np.float32)
    freqs = np.outer(t, inv_freq).astype(np.float32)
    emb = np.concatenate([freqs, freqs], axis=-1)
    cosT = np.ascontiguousarray(np.cos(emb).T.astype(np.float32))
    sinT = np.sin(emb).T.astype(np.float32)
    sinTs = np.ascontiguousarray(np.concatenate([-sinT[:64], sinT[64:]], 0))

    kk = np.arange(P)[:, None]
    qq = np.arange(P)[None, :]
    tri = np.where(qq >= kk, 0.0, NEG).astype(np.float32)

    scale = np.float32(1.0 / math.sqrt(HD))
    in_maps = []
    for c in range(N_CORES):
        r0 = c * D_LOC
        wq = Wqkv[r0:r0 + D_LOC] * scale
        wk = Wqkv[H + r0:H + r0 + D_LOC]
        wv = Wqkv[2 * H + r0:2 * H + r0 + D_LOC]
        wT_c = np.ascontiguousarray(np.concatenate([wq, wk, wv], 0).T)
        woT_c = np.ascontiguousarray(Wo[:, r0:r0 + D_LOC].T)
        in_maps.append({
            "xT": xT, "wT": wT_c, "woT": woT_c,
            "cosT": cosT, "sinTs": sinTs, "tri": tri,
            "ones": np.ones((P, P), dtype=np.float32),
        })
    return in_maps


def kernel(x, Wqkv, Wo):
    nc = _get_nc()
    in_maps = _host_inputs(x, Wqkv, Wo)
    res = run_bass_kernel_spmd(nc, in_maps, list(range(N_CORES)))
    y = res.results[0]["y"].astype(np.float64)
    for c in range(1, N_CORES):
        y += res.results[c]["y"]
    return y.astype(np.float32).reshape(B, L, H)

